# revision 10
# baseline (speedup 1.0000x reference)
"""Multi-head attention (B=4, S=2048, D=1024, H=16) on 8 trn2 NeuronCores.

Sharding (v2): data-parallel over batch x 2-way tensor-parallel over heads.
Core c owns batch b = c//2 and heads [8*(c%2), 8*(c%2)+8) (= model dims
[512*(c%2), 512*(c%2)+512)).  Each core: q/k/v projections for its 8 heads
over its batch's 2048 tokens, attention, partial out-projection against its
512 columns of Wo.  Host sums the 2 partial outputs per batch (all-reduce
of the Megatron pattern at gather time).  vs 8-way head-TP this cuts
per-core HBM traffic 4x (4.2MB bf16 in + 8.4MB f32 out).

Per-core kernel:
  - Heads processed in 4 PAIRS.  Score matmuls have contraction = HD = 64,
    so each kt's two heads run as ROW-TILED CONCURRENT matmuls (PE row
    groups 0-63 / 64-127, auto-derived from base partitions) into separate
    PSUM tiles -> ~2x score throughput vs serial 64-contraction matmuls.
  - exp on ScalarE per (kt, head) [128,1024] tile; ACTIVATEs pipeline at
    ~(N+171)/1.2ns, so this costs only ~7% over 2048-wide tiles while
    halving PSUM (2 banks/tile).
  - v projection in FLIPPED layout (lhsT = x chunk, rhs = Wv): v lands
    [tokens, dims] directly -- zero PE transposes.  A ones column per head
    in v_ext makes attn@v emit numerator + softmax denominator together.
  - attn@v per (head, qc-half, kt-half): 8-matmul groups into one rotating
    PSUM bank, DVE-evicted/accumulated into SBUF f32.  The kt-split lets
    the first half run inside its own stretch, halving exp liveness.
  - division: DVE fast-reciprocal of the denominator row, GPSIMD
    partition_broadcast to 64 rows, DVE multiply into outT (bf16).
  - out-proj contracts all 512 head dims (all pairs) -> runs late; f32
    [2048, 1024] partials out, host adds core pairs.

PSUM (8 banks): scores 3x[128,1024] (6) + attnv 1x[128,512] + shared
proj/oproj 1x[128,512].  An emission-order scheduler pumps an urgent queue
(attnv/divisions -- free exp tiles + PSUM) and a background queue
(projections, out-proj) between score/exp emissions to keep the PE dense
(HAM stays warm) while ScalarE paces the pipeline.
"""
import os
import sys

sys.path.insert(0, "/opt/trn_rl_repo")

from collections import deque
from contextlib import ExitStack

import numpy as np
import ml_dtypes

import concourse.mybir as mybir
import concourse.tile as tile
from concourse import bacc
from concourse._compat import with_exitstack
from concourse.bass_utils import run_bass_kernel_spmd

B, S, D, H = 4, 2048, 1024, 16
HD = D // H              # 64
P = 128
NCORES = 8
ET = D // P              # 8 contraction e-tiles
NPAIR = 4                # head pairs per core (8 heads)
KT = S // P              # 16 key tiles
QC = 1024                # query chunk (stretch granularity)
NQC = S // QC            # 2
TC = 512                 # q/k projection token chunk
VW = 8 * (HD + 1)        # v_ext cols per kt = 520
EXP_SCALE = float(1.0 / np.sqrt(HD))
EPOOL_BUFS = 27

f32 = mybir.dt.float32
bf16 = mybir.dt.bfloat16
Exp = mybir.ActivationFunctionType.Exp

LAST_EXEC_TIME_NS = None
_CACHED_NC = None


@with_exitstack
def _mha_kernel(ctx: ExitStack, tc_: tile.TileContext, ins, outs):
    nc = tc_.nc
    xt_d, wq_d, wk_d, wv_d, wo_d = ins
    out_d = outs[0]

    xpool = ctx.enter_context(tc_.tile_pool(name="xpool", bufs=1))
    wpool = ctx.enter_context(tc_.tile_pool(name="wpool", bufs=1))
    qkpool = ctx.enter_context(tc_.tile_pool(name="qkpool", bufs=2))
    vxpool = ctx.enter_context(tc_.tile_pool(name="vxpool", bufs=1))
    opool = ctx.enter_context(tc_.tile_pool(name="opool", bufs=1))
    ocpool = ctx.enter_context(tc_.tile_pool(name="ocpool", bufs=2))
    dpool = ctx.enter_context(tc_.tile_pool(name="dpool", bufs=2))
    rbpool = ctx.enter_context(tc_.tile_pool(name="rbpool", bufs=2))
    ospool = ctx.enter_context(tc_.tile_pool(name="ospool", bufs=4))
    epool = ctx.enter_context(tc_.tile_pool(name="epool", bufs=EPOOL_BUFS))

    scp = ctx.enter_context(tc_.tile_pool(name="scp", bufs=3, space="PSUM"))
    oep = ctx.enter_context(tc_.tile_pool(name="oep", bufs=1, space="PSUM"))
    mpp = ctx.enter_context(tc_.tile_pool(name="mpp", bufs=1, space="PSUM"))

    # flat 2D layouts; host pre-arranges to match
    xt = xpool.tile([P, ET * S], bf16, tag="xt")            # [e-tile, tok]
    wq = wpool.tile([P, ET * NPAIR * P], bf16, tag="wq")    # [et, pair, hd]
    wk = wpool.tile([P, ET * NPAIR * P], bf16, tag="wk")
    wv = wpool.tile([P, ET * 512], bf16, tag="wvo")         # slot reused by wo
    v_ext = vxpool.tile([P, KT * VW], bf16, tag="vx")       # [kt, 8h, 65]
    outT = opool.tile([P, NPAIR * S], bf16, tag="outT")     # [hd-tile, tok]

    # pair-striped weight loads + half-chunked first x load: the first
    # PK/PQ groups need only pair-0 weight columns and tokens 0-255
    wk4 = wk[:].rearrange("p (e pr c) -> p e pr c", pr=NPAIR, c=P)
    wkd4 = wk_d[:].rearrange("p (e pr c) -> p e pr c", pr=NPAIR, c=P)
    wq4 = wq[:].rearrange("p (e pr c) -> p e pr c", pr=NPAIR, c=P)
    wqd4 = wq_d[:].rearrange("p (e pr c) -> p e pr c", pr=NPAIR, c=P)
    xt3 = xt[:].rearrange("p (e t) -> p e t", e=ET)
    xd3 = xt_d[:].rearrange("p (e t) -> p e t", e=ET)
    nc.gpsimd.dma_start(wk4[:, :, 0:1, :], wkd4[:, :, 0:1, :])
    nc.gpsimd.dma_start(xt3[:, :, 0:256], xd3[:, :, 0:256])
    nc.gpsimd.dma_start(xt3[:, :, 256:512], xd3[:, :, 256:512])
    nc.gpsimd.dma_start(xt3[:, :, 512:1024], xd3[:, :, 512:1024])
    nc.gpsimd.dma_start(wk4[:, :, 1:4, :], wkd4[:, :, 1:4, :])
    for c in range(2, 4):
        nc.gpsimd.dma_start(xt3[:, :, c * TC:(c + 1) * TC],
                            xd3[:, :, c * TC:(c + 1) * TC])
    nc.sync.dma_start(wq4[:, :, 0:1, :], wqd4[:, :, 0:1, :])
    nc.sync.dma_start(wq4[:, :, 1:4, :], wqd4[:, :, 1:4, :])
    nc.sync.dma_start(wv[:], wv_d[:])

    # ones columns of v_ext (col 64 of each head block), set once
    vcols = v_ext[:].rearrange("p (kh c) -> p kh c", c=HD + 1)
    nc.vector.memset(vcols[:, :, HD:HD + 1], 1.0)

    qTs, kTs = {}, {}
    exps = {}
    oecps, denss = {}, {}
    wo_box = {}

    # ---------------- unit bodies ----------------
    def alloc_qk(p):
        if p not in kTs:
            qTs[p] = qkpool.tile([P, S], bf16, tag="qT", name=f"qT{p}")
            kTs[p] = qkpool.tile([P, S], bf16, tag="kT", name=f"kT{p}")

    def proj_qk(w, dst, p, c, t0=0, tw=TC):
        # one 8-matmul accumulation group: [128 pair-dims, tw tokens]
        pp = mpp.tile([P, TC], f32, tag="mp")
        base = c * TC + t0
        for et in range(ET):
            nc.tensor.matmul(
                pp[0:P, 0:tw],
                w[:, (et * NPAIR + p) * P:(et * NPAIR + p + 1) * P],
                xt[:, et * S + base: et * S + base + tw],
                start=(et == 0), stop=(et == ET - 1),
            )
        nc.vector.tensor_copy(dst[:, base:base + tw], pp[0:P, 0:tw])

    def proj_v(c):
        # flipped: [128 tokens of kt-tile c, 512 v-dims]
        pv = mpp.tile([P, TC], f32, tag="mp")
        for et in range(ET):
            nc.tensor.matmul(
                pv[:],
                xt[:, et * S + c * P: et * S + (c + 1) * P],
                wv[:, et * 512:(et + 1) * 512],
                start=(et == 0), stop=(et == ET - 1),
            )
        dst = v_ext[:, c * VW:(c + 1) * VW].rearrange(
            "p (h c2) -> p h c2", c2=HD + 1)[:, :, 0:HD]
        nc.vector.tensor_copy(dst, pv[:].rearrange("p (h c2) -> p h c2", c2=HD))

    def load_wo():
        wo = wpool.tile([P, NPAIR * D], bf16, tag="wvo", name="wo")
        nc.sync.dma_start(wo[:], wo_d[:])
        wo_box["wo"] = wo

    def scores_unit(p, qc, kt):
        # row-tiled concurrent head pair: h0 rows 0-63, h1 rows 64-127
        sc0 = scp.tile([P, QC], f32, tag="sc", name=f"sc{p}_{qc}_{kt}_0")
        sc1 = scp.tile([P, QC], f32, tag="sc", name=f"sc{p}_{qc}_{kt}_1")
        kTp, qTp = kTs[p], qTs[p]
        for l in range(2):
            for h, sc in ((0, sc0), (1, sc1)):
                rows = slice(h * HD, (h + 1) * HD)
                nc.tensor.matmul(
                    sc[:, l * 512:(l + 1) * 512],
                    kTp[rows, kt * P:(kt + 1) * P],
                    qTp[rows, qc * QC + l * 512: qc * QC + (l + 1) * 512],
                    start=True, stop=True,
                )
        for h, sc in ((0, sc0), (1, sc1)):
            ex = epool.tile([P, QC], bf16, tag="exp", name=f"ex{p}_{qc}_{kt}_{h}")
            nc.scalar.activation(ex[:], sc[:], Exp, scale=EXP_SCALE)
            exps[(p, qc, kt, h)] = ex

    def attnv_unit(p, qc, h, l, kh):
        # one contiguous 8-matmul accumulation group over kt half kh
        key = (p, qc, h)
        if key not in oecps:
            oecps[key] = ocpool.tile([P, QC], f32, tag="ocp",
                                     name=f"ocp{p}_{qc}_{h}")
            denss[key] = dpool.tile([1, QC], f32, tag="dens",
                                    name=f"den{p}_{qc}_{h}")
        oe = oep.tile([P, 512], f32, tag="oe")
        base = (2 * p + h) * (HD + 1)
        for i in range(8):
            kt = kh * 8 + i
            nc.tensor.matmul(
                oe[0:HD + 1, :],
                v_ext[:, kt * VW + base: kt * VW + base + HD + 1],
                exps[(p, qc, kt, h)][:, l * 512:(l + 1) * 512],
                start=(i == 0), stop=(i == 7),
            )
        ocp, dn = oecps[key], denss[key]
        ls = slice(l * 512, (l + 1) * 512)
        if kh == 0:
            nc.vector.tensor_copy(ocp[0:HD, ls], oe[0:HD, :])
            nc.vector.tensor_copy(dn[0:1, ls], oe[HD:HD + 1, :])
        else:
            nc.vector.tensor_add(ocp[0:HD, ls], ocp[0:HD, ls], oe[0:HD, :])
            nc.vector.tensor_add(dn[0:1, ls], dn[0:1, ls], oe[HD:HD + 1, :])

    def div_unit(p, qc, h):
        key = (p, qc, h)
        rec = dpool.tile([1, QC], f32, tag="recs", name=f"rec{p}_{qc}_{h}")
        nc.vector.reciprocal_approx_fast(rec[:], denss[key][:])
        rb = rbpool.tile([HD, QC], f32, tag="rb", name=f"rb{p}_{qc}_{h}")
        nc.gpsimd.partition_broadcast(rb[:], rec[:])
        dst = outT[h * HD:(h + 1) * HD, p * S + qc * QC: p * S + (qc + 1) * QC]
        nc.vector.tensor_mul(dst, oecps[key][0:HD, :], rb[:])

    in_drain = [False]

    def warm_unit():
        dmp = mpp.tile([P, TC], f32, tag="mp")
        nc.tensor.matmul(dmp[:], v_ext[:, 0:P], v_ext[:, 0:TC],
                         start=True, stop=True)

    def oproj_unit(qc, tc):
        t0 = (qc * 8 + tc) * P
        wo = wo_box["wo"]
        osb = ospool.tile([P, D], f32, tag="osb")
        for eh in range(2):
            po = (scp.tile([P, QC], f32, tag="sc", name=f"po{qc}_{tc}_{eh}")
                  if in_drain[0] else mpp.tile([P, TC], f32, tag="mp"))
            for ht in range(NPAIR):
                nc.tensor.matmul(
                    po[0:P, 0:TC],
                    outT[:, ht * S + t0: ht * S + t0 + P],
                    wo[:, ht * D + eh * 512: ht * D + (eh + 1) * 512],
                    start=(ht == 0), stop=(ht == NPAIR - 1),
                )
            nc.vector.tensor_copy(osb[:, eh * 512:(eh + 1) * 512],
                                  po[0:P, 0:TC])
        eng = nc.sync if tc % 2 == 0 else nc.gpsimd
        eng.dma_start(out_d[t0:t0 + P, :], osb[:])

    # ---------------- scheduler ----------------
    urgent = deque()   # (fn, cycles)
    backg = deque()    # (fn, cycles, ready_si, tag)
    cur_si = 0
    budget = 0.0

    def pump(room):
        nonlocal budget
        budget = min(budget + room, 6000.0)
        while budget > 0:
            if urgent:
                fn, cyc = urgent.popleft()
            elif backg and backg[0][2] <= cur_si:
                fn, cyc, _, _ = backg.popleft()
            else:
                break
            fn()
            budget -= cyc

    def pump_until(tag):
        # emit queued units in order until no `tag` units remain in backg
        while any(t == tag for _, _, _, t in backg):
            if urgent:
                fn, cyc = urgent.popleft()
            else:
                fn, cyc, _, _ = backg.popleft()
            fn()

    # ---------------- lead-in ----------------
    alloc_qk(0)
    proj_qk(wk, kTs[0], 0, 0, 0, 256)
    proj_qk(wq, qTs[0], 0, 0, 0, 256)
    proj_qk(wk, kTs[0], 0, 0, 256, 256)
    proj_qk(wq, qTs[0], 0, 0, 256, 256)
    proj_qk(wq, qTs[0], 0, 1)

    pv_unit = lambda cc: ((lambda: proj_v(cc)), 4400, 0,
                          "pv0" if cc < 8 else "pv1")
    backg.append((lambda: proj_qk(wk, kTs[0], 0, 1), 4400, 0, "kq0"))
    for c in range(0, 4):
        backg.append(pv_unit(c))
    backg.append((lambda: proj_qk(wk, kTs[0], 0, 2), 4400, 0, "kq0"))
    for c in range(4, 8):
        backg.append(pv_unit(c))
    backg.append((lambda: proj_qk(wk, kTs[0], 0, 3), 4400, 0, "kq0"))
    backg.append((lambda: proj_qk(wq, qTs[0], 0, 2), 4400, 0, "kq0"))
    backg.append((lambda: proj_qk(wq, qTs[0], 0, 3), 4400, 0, "kq0"))
    for c in range(8, KT):
        backg.append(pv_unit(c))
    backg.append((load_wo, 100, 0, "wo"))
    for p in range(1, NPAIR):
        ready = 2 * p - 1
        for c in range(4):
            backg.append((
                (lambda pp, cc: lambda: (alloc_qk(pp),
                                         proj_qk(wk, kTs[pp], pp, cc))[-1])(p, c),
                4400, ready, f"kq{p}"))
        for c in range(4):
            backg.append((
                (lambda pp, cc: lambda: proj_qk(wq, qTs[pp], pp, cc))(p, c),
                4400, ready, f"kq{p}"))

    # ---------------- stretches ----------------
    stretches = [(p, qc) for p in range(NPAIR) for qc in range(NQC)]
    for si, (p, qc) in enumerate(stretches):
        cur_si = si
        if qc == 0 and p > 0:
            pump_until(f"kq{p}")   # scores(p) need qT/kT(p) emitted first
        for kt in range(KT):
            scores_unit(p, qc, kt)
            if kt == 7 and si == 0:
                pump_until("pv0")  # attnv kt 0-7 needs v_ext chunks 0-7
            if kt in (7, 9, 11, 13):
                h, l = divmod((kt - 7) // 2, 2)
                urgent.append((
                    (lambda a, b, c2, d: lambda: attnv_unit(a, b, c2, d, 0)
                     )(p, qc, h, l), 4400))
            pump(4600)
        # second kt-halves + divisions, consumed during the next stretch
        if si == 0:
            pump_until("pv1")  # attnv kt 8-15 needs v_ext chunks 8-15
        for h in range(2):
            for l in range(2):
                urgent.append((
                    (lambda a, b, c2, d: lambda: attnv_unit(a, b, c2, d, 1)
                     )(p, qc, h, l), 4400))
            urgent.append((
                (lambda a, b, c2: lambda: div_unit(a, b, c2))(p, qc, h), 600))
        if p == NPAIR - 1:
            # out-proj for this qc: available once p3's divisions (just
            # queued ahead of these in-order) have been emitted
            for tc in range(8):
                backg.append((
                    (lambda q2, t2: lambda: oproj_unit(q2, t2))(qc, tc),
                    4800, si, "po"))

    for _ in range(10):
        urgent.append((warm_unit, 550))
    cur_si = len(stretches)
    in_drain[0] = True
    while urgent or backg:
        pump(10000)


def _build():
    global _CACHED_NC
    if _CACHED_NC is not None:
        return _CACHED_NC
    nc = bacc.Bacc("TRN2", target_bir_lowering=False, debug=False)
    xt = nc.dram_tensor("xt", [P, ET * S], bf16, kind="ExternalInput").ap()
    wq = nc.dram_tensor("wq", [P, ET * NPAIR * P], bf16,
                        kind="ExternalInput").ap()
    wk = nc.dram_tensor("wk", [P, ET * NPAIR * P], bf16,
                        kind="ExternalInput").ap()
    wv = nc.dram_tensor("wv", [P, ET * 512], bf16, kind="ExternalInput").ap()
    wo = nc.dram_tensor("wo", [P, NPAIR * D], bf16, kind="ExternalInput").ap()
    out = nc.dram_tensor("out", [S, D], f32, kind="ExternalOutput").ap()

    with tile.TileContext(nc) as tc_:
        _mha_kernel(tc_, [xt, wq, wk, wv, wo], [out])
    nc.compile()
    _CACHED_NC = nc
    return nc


def kernel(x: np.ndarray, Wq: np.ndarray, Wk: np.ndarray, Wv: np.ndarray,
           Wo: np.ndarray) -> np.ndarray:
    global LAST_EXEC_TIME_NS
    nc = _build()
    bf = ml_dtypes.bfloat16

    x = np.asarray(x, dtype=np.float32)
    Wq = np.asarray(Wq, np.float32)
    Wk = np.asarray(Wk, np.float32)
    Wv = np.asarray(Wv, np.float32)
    Wo = np.asarray(Wo, np.float32)

    in_maps = []
    for c in range(NCORES):
        b, tp = c // 2, c % 2
        hs = tp * 512
        # xt: [D, S] -> [et, 128, S] -> [128, et*S]
        xt = np.ascontiguousarray(
            x[b].T.reshape(ET, P, S).transpose(1, 0, 2)).astype(bf)
        # wq/wk: W[hs:hs+512, :].T = [e, hd] -> [et, 128, pair, 128] -> p-first
        wq = np.ascontiguousarray(
            Wq[hs:hs + 512, :].T.reshape(ET, P, NPAIR, P)
            .transpose(1, 0, 2, 3)).astype(bf)
        wk = np.ascontiguousarray(
            Wk[hs:hs + 512, :].T.reshape(ET, P, NPAIR, P)
            .transpose(1, 0, 2, 3)).astype(bf)
        wv = np.ascontiguousarray(
            Wv[hs:hs + 512, :].T.reshape(ET, P, 512)
            .transpose(1, 0, 2)).astype(bf)
        # wo: Wo[:, hs:hs+512].T = [hd, e] -> [hdtile, 128, 1024] -> p-first
        wo = np.ascontiguousarray(
            Wo[:, hs:hs + 512].T.reshape(NPAIR, P, D)
            .transpose(1, 0, 2)).astype(bf)
        in_maps.append({
            "xt": xt.reshape(P, ET * S),
            "wq": wq.reshape(P, ET * NPAIR * P),
            "wk": wk.reshape(P, ET * NPAIR * P),
            "wv": wv.reshape(P, ET * 512),
            "wo": wo.reshape(P, NPAIR * D),
        })

    trace = bool(os.environ.get("BASS_TRACE"))
    res = run_bass_kernel_spmd(nc, in_maps, core_ids=list(range(NCORES)),
                               trace=trace)
    LAST_EXEC_TIME_NS = res.exec_time_ns

    outs = [np.asarray(r["out"], np.float32) for r in res.results]
    return np.stack([outs[2 * b] + outs[2 * b + 1] for b in range(B)])


# revision 11
# speedup vs baseline: 1.0016x; 1.0016x over previous
"""Multi-head attention (B=4, S=2048, D=1024, H=16) on 8 trn2 NeuronCores.

Sharding (v2): data-parallel over batch x 2-way tensor-parallel over heads.
Core c owns batch b = c//2 and heads [8*(c%2), 8*(c%2)+8) (= model dims
[512*(c%2), 512*(c%2)+512)).  Each core: q/k/v projections for its 8 heads
over its batch's 2048 tokens, attention, partial out-projection against its
512 columns of Wo.  Host sums the 2 partial outputs per batch (all-reduce
of the Megatron pattern at gather time).  vs 8-way head-TP this cuts
per-core HBM traffic 4x (4.2MB bf16 in + 8.4MB f32 out).

Per-core kernel:
  - Heads processed in 4 PAIRS.  Score matmuls have contraction = HD = 64,
    so each kt's two heads run as ROW-TILED CONCURRENT matmuls (PE row
    groups 0-63 / 64-127, auto-derived from base partitions) into separate
    PSUM tiles -> ~2x score throughput vs serial 64-contraction matmuls.
  - exp on ScalarE per (kt, head) [128,1024] tile; ACTIVATEs pipeline at
    ~(N+171)/1.2ns, so this costs only ~7% over 2048-wide tiles while
    halving PSUM (2 banks/tile).
  - v projection in FLIPPED layout (lhsT = x chunk, rhs = Wv): v lands
    [tokens, dims] directly -- zero PE transposes.  A ones column per head
    in v_ext makes attn@v emit numerator + softmax denominator together.
  - attn@v per (head, qc-half, kt-half): 8-matmul groups into one rotating
    PSUM bank, DVE-evicted/accumulated into SBUF f32.  The kt-split lets
    the first half run inside its own stretch, halving exp liveness.
  - division: DVE fast-reciprocal of the denominator row, GPSIMD
    partition_broadcast to 64 rows, DVE multiply into outT (bf16).
  - out-proj contracts all 512 head dims (all pairs) -> runs late; f32
    [2048, 1024] partials out, host adds core pairs.

PSUM (8 banks): scores 3x[128,1024] (6) + attnv 1x[128,512] + shared
proj/oproj 1x[128,512].  An emission-order scheduler pumps an urgent queue
(attnv/divisions -- free exp tiles + PSUM) and a background queue
(projections, out-proj) between score/exp emissions to keep the PE dense
(HAM stays warm) while ScalarE paces the pipeline.
"""
import os
import sys

sys.path.insert(0, "/opt/trn_rl_repo")

from collections import deque
from contextlib import ExitStack

import numpy as np
import ml_dtypes

import concourse.mybir as mybir
import concourse.tile as tile
from concourse import bacc
from concourse._compat import with_exitstack
from concourse.bass_utils import run_bass_kernel_spmd

B, S, D, H = 4, 2048, 1024, 16
HD = D // H              # 64
P = 128
NCORES = 8
ET = D // P              # 8 contraction e-tiles
NPAIR = 4                # head pairs per core (8 heads)
KT = S // P              # 16 key tiles
QC = 1024                # query chunk (stretch granularity)
NQC = S // QC            # 2
TC = 512                 # q/k projection token chunk
VW = 8 * (HD + 1)        # v_ext cols per kt = 520
EXP_SCALE = float(1.0 / np.sqrt(HD))
EPOOL_BUFS = 27

f32 = mybir.dt.float32
bf16 = mybir.dt.bfloat16
Exp = mybir.ActivationFunctionType.Exp

LAST_EXEC_TIME_NS = None
_CACHED_NC = None


@with_exitstack
def _mha_kernel(ctx: ExitStack, tc_: tile.TileContext, ins, outs):
    nc = tc_.nc
    xt_d, wq_d, wk_d, wv_d, wo_d = ins
    out_d = outs[0]

    xpool = ctx.enter_context(tc_.tile_pool(name="xpool", bufs=1))
    wpool = ctx.enter_context(tc_.tile_pool(name="wpool", bufs=1))
    qkpool = ctx.enter_context(tc_.tile_pool(name="qkpool", bufs=2))
    vxpool = ctx.enter_context(tc_.tile_pool(name="vxpool", bufs=1))
    opool = ctx.enter_context(tc_.tile_pool(name="opool", bufs=1))
    ocpool = ctx.enter_context(tc_.tile_pool(name="ocpool", bufs=2))
    dpool = ctx.enter_context(tc_.tile_pool(name="dpool", bufs=2))
    rbpool = ctx.enter_context(tc_.tile_pool(name="rbpool", bufs=2))
    ospool = ctx.enter_context(tc_.tile_pool(name="ospool", bufs=4))
    epool = ctx.enter_context(tc_.tile_pool(name="epool", bufs=EPOOL_BUFS))

    scp = ctx.enter_context(tc_.tile_pool(name="scp", bufs=3, space="PSUM"))
    oep = ctx.enter_context(tc_.tile_pool(name="oep", bufs=1, space="PSUM"))
    mpp = ctx.enter_context(tc_.tile_pool(name="mpp", bufs=1, space="PSUM"))

    # flat 2D layouts; host pre-arranges to match
    xt = xpool.tile([P, ET * S], bf16, tag="xt")            # [e-tile, tok]
    wq = wpool.tile([P, ET * NPAIR * P], bf16, tag="wq")    # [et, pair, hd]
    wk = wpool.tile([P, ET * NPAIR * P], bf16, tag="wk")
    wv = wpool.tile([P, ET * 512], bf16, tag="wvo")         # slot reused by wo
    v_ext = vxpool.tile([P, KT * VW], bf16, tag="vx")       # [kt, 8h, 65]
    outT = opool.tile([P, NPAIR * S], bf16, tag="outT")     # [hd-tile, tok]

    # pair-striped weight loads + half-chunked first x load: the first
    # PK/PQ groups need only pair-0 weight columns and tokens 0-255
    wk4 = wk[:].rearrange("p (e pr c) -> p e pr c", pr=NPAIR, c=P)
    wkd4 = wk_d[:].rearrange("p (e pr c) -> p e pr c", pr=NPAIR, c=P)
    wq4 = wq[:].rearrange("p (e pr c) -> p e pr c", pr=NPAIR, c=P)
    wqd4 = wq_d[:].rearrange("p (e pr c) -> p e pr c", pr=NPAIR, c=P)
    xt3 = xt[:].rearrange("p (e t) -> p e t", e=ET)
    xd3 = xt_d[:].rearrange("p (e t) -> p e t", e=ET)
    nc.gpsimd.dma_start(wk4[:, :, 0:1, :], wkd4[:, :, 0:1, :])
    nc.gpsimd.dma_start(xt3[:, :, 0:256], xd3[:, :, 0:256])
    nc.gpsimd.dma_start(xt3[:, :, 256:512], xd3[:, :, 256:512])
    nc.gpsimd.dma_start(xt3[:, :, 512:1024], xd3[:, :, 512:1024])
    nc.gpsimd.dma_start(wk4[:, :, 1:4, :], wkd4[:, :, 1:4, :])
    for c in range(2, 4):
        nc.gpsimd.dma_start(xt3[:, :, c * TC:(c + 1) * TC],
                            xd3[:, :, c * TC:(c + 1) * TC])
    nc.sync.dma_start(wq4[:, :, 0:1, :], wqd4[:, :, 0:1, :])
    nc.sync.dma_start(wq4[:, :, 1:4, :], wqd4[:, :, 1:4, :])
    nc.sync.dma_start(wv[:], wv_d[:])

    # ones columns of v_ext (col 64 of each head block), set once
    vcols = v_ext[:].rearrange("p (kh c) -> p kh c", c=HD + 1)
    nc.vector.memset(vcols[:, :, HD:HD + 1], 1.0)

    qTs, kTs = {}, {}
    exps = {}
    oecps, denss = {}, {}
    wo_box = {}

    # ---------------- unit bodies ----------------
    def alloc_qk(p):
        if p not in kTs:
            qTs[p] = qkpool.tile([P, S], bf16, tag="qT", name=f"qT{p}")
            kTs[p] = qkpool.tile([P, S], bf16, tag="kT", name=f"kT{p}")

    def proj_qk(w, dst, p, c, t0=0, tw=TC):
        # one 8-matmul accumulation group: [128 pair-dims, tw tokens]
        pp = mpp.tile([P, TC], f32, tag="mp")
        base = c * TC + t0
        for et in range(ET):
            nc.tensor.matmul(
                pp[0:P, 0:tw],
                w[:, (et * NPAIR + p) * P:(et * NPAIR + p + 1) * P],
                xt[:, et * S + base: et * S + base + tw],
                start=(et == 0), stop=(et == ET - 1),
            )
        nc.vector.tensor_copy(dst[:, base:base + tw], pp[0:P, 0:tw])

    def proj_v(c):
        # flipped: [128 tokens of kt-tile c, 512 v-dims]
        pv = mpp.tile([P, TC], f32, tag="mp")
        for et in range(ET):
            nc.tensor.matmul(
                pv[:],
                xt[:, et * S + c * P: et * S + (c + 1) * P],
                wv[:, et * 512:(et + 1) * 512],
                start=(et == 0), stop=(et == ET - 1),
            )
        dst = v_ext[:, c * VW:(c + 1) * VW].rearrange(
            "p (h c2) -> p h c2", c2=HD + 1)[:, :, 0:HD]
        nc.vector.tensor_copy(dst, pv[:].rearrange("p (h c2) -> p h c2", c2=HD))

    def load_wo():
        wo = wpool.tile([P, NPAIR * D], bf16, tag="wvo", name="wo")
        nc.sync.dma_start(wo[:], wo_d[:])
        wo_box["wo"] = wo

    def scores_unit(p, qc, kt):
        # row-tiled concurrent head pair: h0 rows 0-63, h1 rows 64-127
        sc0 = scp.tile([P, QC], f32, tag="sc", name=f"sc{p}_{qc}_{kt}_0")
        sc1 = scp.tile([P, QC], f32, tag="sc", name=f"sc{p}_{qc}_{kt}_1")
        kTp, qTp = kTs[p], qTs[p]
        for l in range(2):
            for h, sc in ((0, sc0), (1, sc1)):
                rows = slice(h * HD, (h + 1) * HD)
                nc.tensor.matmul(
                    sc[:, l * 512:(l + 1) * 512],
                    kTp[rows, kt * P:(kt + 1) * P],
                    qTp[rows, qc * QC + l * 512: qc * QC + (l + 1) * 512],
                    start=True, stop=True,
                )
        for h, sc in ((0, sc0), (1, sc1)):
            ex = epool.tile([P, QC], bf16, tag="exp", name=f"ex{p}_{qc}_{kt}_{h}")
            nc.scalar.activation(ex[:], sc[:], Exp, scale=EXP_SCALE)
            exps[(p, qc, kt, h)] = ex

    def attnv_unit(p, qc, h, l, kh):
        # one contiguous 8-matmul accumulation group over kt half kh
        key = (p, qc, h)
        if key not in oecps:
            oecps[key] = ocpool.tile([P, QC], f32, tag="ocp",
                                     name=f"ocp{p}_{qc}_{h}")
            denss[key] = dpool.tile([1, QC], f32, tag="dens",
                                    name=f"den{p}_{qc}_{h}")
        oe = oep.tile([P, 512], f32, tag="oe")
        base = (2 * p + h) * (HD + 1)
        for i in range(8):
            kt = kh * 8 + i
            nc.tensor.matmul(
                oe[0:HD + 1, :],
                v_ext[:, kt * VW + base: kt * VW + base + HD + 1],
                exps[(p, qc, kt, h)][:, l * 512:(l + 1) * 512],
                start=(i == 0), stop=(i == 7),
            )
        ocp, dn = oecps[key], denss[key]
        ls = slice(l * 512, (l + 1) * 512)
        if kh == 0:
            nc.vector.tensor_copy(ocp[0:HD, ls], oe[0:HD, :])
            nc.vector.tensor_copy(dn[0:1, ls], oe[HD:HD + 1, :])
        else:
            nc.vector.tensor_add(ocp[0:HD, ls], ocp[0:HD, ls], oe[0:HD, :])
            nc.vector.tensor_add(dn[0:1, ls], dn[0:1, ls], oe[HD:HD + 1, :])

    def div_unit(p, qc, h):
        key = (p, qc, h)
        rec = dpool.tile([1, QC], f32, tag="recs", name=f"rec{p}_{qc}_{h}")
        nc.vector.reciprocal_approx_fast(rec[:], denss[key][:])
        rb = rbpool.tile([HD, QC], f32, tag="rb", name=f"rb{p}_{qc}_{h}")
        nc.gpsimd.partition_broadcast(rb[:], rec[:])
        dst = outT[h * HD:(h + 1) * HD, p * S + qc * QC: p * S + (qc + 1) * QC]
        nc.vector.tensor_mul(dst, oecps[key][0:HD, :], rb[:])

    in_drain = [False]

    def oproj_unit(qc, tc):
        t0 = (qc * 8 + tc) * P
        wo = wo_box["wo"]
        osb = ospool.tile([P, D], f32, tag="osb")
        for eh in range(2):
            po = (scp.tile([P, QC], f32, tag="sc", name=f"po{qc}_{tc}_{eh}")
                  if in_drain[0] else mpp.tile([P, TC], f32, tag="mp"))
            for ht in range(NPAIR):
                nc.tensor.matmul(
                    po[0:P, 0:TC],
                    outT[:, ht * S + t0: ht * S + t0 + P],
                    wo[:, ht * D + eh * 512: ht * D + (eh + 1) * 512],
                    start=(ht == 0), stop=(ht == NPAIR - 1),
                )
            nc.vector.tensor_copy(osb[:, eh * 512:(eh + 1) * 512],
                                  po[0:P, 0:TC])
        eng = nc.sync if tc % 2 == 0 else nc.gpsimd
        eng.dma_start(out_d[t0:t0 + P, :], osb[:])

    # ---------------- scheduler ----------------
    urgent = deque()   # (fn, cycles)
    backg = deque()    # (fn, cycles, ready_si, tag)
    cur_si = 0
    budget = 0.0

    def pump(room):
        nonlocal budget
        budget = min(budget + room, 6000.0)
        while budget > 0:
            if urgent:
                fn, cyc = urgent.popleft()
            elif backg and backg[0][2] <= cur_si:
                fn, cyc, _, _ = backg.popleft()
            else:
                break
            fn()
            budget -= cyc

    def pump_until(tag):
        # emit queued units in order until no `tag` units remain in backg
        while any(t == tag for _, _, _, t in backg):
            if urgent:
                fn, cyc = urgent.popleft()
            else:
                fn, cyc, _, _ = backg.popleft()
            fn()

    # ---------------- lead-in ----------------
    alloc_qk(0)
    proj_qk(wk, kTs[0], 0, 0, 0, 256)
    proj_qk(wq, qTs[0], 0, 0, 0, 256)
    proj_qk(wk, kTs[0], 0, 0, 256, 256)
    proj_qk(wq, qTs[0], 0, 0, 256, 256)
    proj_qk(wq, qTs[0], 0, 1)

    pv_unit = lambda cc: ((lambda: proj_v(cc)), 4400, 0,
                          "pv0" if cc < 8 else "pv1")
    backg.append((lambda: proj_qk(wk, kTs[0], 0, 1), 4400, 0, "kq0"))
    for c in range(0, 4):
        backg.append(pv_unit(c))
    backg.append((lambda: proj_qk(wk, kTs[0], 0, 2), 4400, 0, "kq0"))
    for c in range(4, 8):
        backg.append(pv_unit(c))
    backg.append((lambda: proj_qk(wk, kTs[0], 0, 3), 4400, 0, "kq0"))
    backg.append((lambda: proj_qk(wq, qTs[0], 0, 2), 4400, 0, "kq0"))
    backg.append((lambda: proj_qk(wq, qTs[0], 0, 3), 4400, 0, "kq0"))
    for c in range(8, KT):
        backg.append(pv_unit(c))
    backg.append((load_wo, 100, 0, "wo"))
    for p in range(1, NPAIR):
        ready = 2 * p - 1
        for c in range(4):
            backg.append((
                (lambda pp, cc: lambda: (alloc_qk(pp),
                                         proj_qk(wk, kTs[pp], pp, cc))[-1])(p, c),
                4400, ready, f"kq{p}"))
        for c in range(4):
            backg.append((
                (lambda pp, cc: lambda: proj_qk(wq, qTs[pp], pp, cc))(p, c),
                4400, ready, f"kq{p}"))

    # ---------------- stretches ----------------
    stretches = [(p, qc) for p in range(NPAIR) for qc in range(NQC)]
    for si, (p, qc) in enumerate(stretches):
        cur_si = si
        if qc == 0 and p > 0:
            pump_until(f"kq{p}")   # scores(p) need qT/kT(p) emitted first
        for kt in range(KT):
            scores_unit(p, qc, kt)
            if kt == 7 and si == 0:
                pump_until("pv0")  # attnv kt 0-7 needs v_ext chunks 0-7
            if kt in (7, 9, 11, 13):
                h, l = divmod((kt - 7) // 2, 2)
                urgent.append((
                    (lambda a, b, c2, d: lambda: attnv_unit(a, b, c2, d, 0)
                     )(p, qc, h, l), 4400))
            pump(4600)
        # second kt-halves + divisions, consumed during the next stretch
        if si == 0:
            pump_until("pv1")  # attnv kt 8-15 needs v_ext chunks 8-15
        for h in range(2):
            for l in range(2):
                urgent.append((
                    (lambda a, b, c2, d: lambda: attnv_unit(a, b, c2, d, 1)
                     )(p, qc, h, l), 4400))
            urgent.append((
                (lambda a, b, c2: lambda: div_unit(a, b, c2))(p, qc, h), 600))
        if p == NPAIR - 1:
            # out-proj for this qc: available once p3's divisions (just
            # queued ahead of these in-order) have been emitted
            for tc in range(8):
                backg.append((
                    (lambda q2, t2: lambda: oproj_unit(q2, t2))(qc, tc),
                    4800, si, "po"))

    cur_si = len(stretches)
    in_drain[0] = True
    while urgent or backg:
        pump(10000)


def _build():
    global _CACHED_NC
    if _CACHED_NC is not None:
        return _CACHED_NC
    nc = bacc.Bacc("TRN2", target_bir_lowering=False, debug=False)
    xt = nc.dram_tensor("xt", [P, ET * S], bf16, kind="ExternalInput").ap()
    wq = nc.dram_tensor("wq", [P, ET * NPAIR * P], bf16,
                        kind="ExternalInput").ap()
    wk = nc.dram_tensor("wk", [P, ET * NPAIR * P], bf16,
                        kind="ExternalInput").ap()
    wv = nc.dram_tensor("wv", [P, ET * 512], bf16, kind="ExternalInput").ap()
    wo = nc.dram_tensor("wo", [P, NPAIR * D], bf16, kind="ExternalInput").ap()
    out = nc.dram_tensor("out", [S, D], f32, kind="ExternalOutput").ap()

    with tile.TileContext(nc) as tc_:
        _mha_kernel(tc_, [xt, wq, wk, wv, wo], [out])
    nc.compile()
    _CACHED_NC = nc
    return nc


def kernel(x: np.ndarray, Wq: np.ndarray, Wk: np.ndarray, Wv: np.ndarray,
           Wo: np.ndarray) -> np.ndarray:
    global LAST_EXEC_TIME_NS
    nc = _build()
    bf = ml_dtypes.bfloat16

    x = np.asarray(x, dtype=np.float32)
    Wq = np.asarray(Wq, np.float32)
    Wk = np.asarray(Wk, np.float32)
    Wv = np.asarray(Wv, np.float32)
    Wo = np.asarray(Wo, np.float32)

    in_maps = []
    for c in range(NCORES):
        b, tp = c // 2, c % 2
        hs = tp * 512
        # xt: [D, S] -> [et, 128, S] -> [128, et*S]
        xt = np.ascontiguousarray(
            x[b].T.reshape(ET, P, S).transpose(1, 0, 2)).astype(bf)
        # wq/wk: W[hs:hs+512, :].T = [e, hd] -> [et, 128, pair, 128] -> p-first
        wq = np.ascontiguousarray(
            Wq[hs:hs + 512, :].T.reshape(ET, P, NPAIR, P)
            .transpose(1, 0, 2, 3)).astype(bf)
        wk = np.ascontiguousarray(
            Wk[hs:hs + 512, :].T.reshape(ET, P, NPAIR, P)
            .transpose(1, 0, 2, 3)).astype(bf)
        wv = np.ascontiguousarray(
            Wv[hs:hs + 512, :].T.reshape(ET, P, 512)
            .transpose(1, 0, 2)).astype(bf)
        # wo: Wo[:, hs:hs+512].T = [hd, e] -> [hdtile, 128, 1024] -> p-first
        wo = np.ascontiguousarray(
            Wo[:, hs:hs + 512].T.reshape(NPAIR, P, D)
            .transpose(1, 0, 2)).astype(bf)
        in_maps.append({
            "xt": xt.reshape(P, ET * S),
            "wq": wq.reshape(P, ET * NPAIR * P),
            "wk": wk.reshape(P, ET * NPAIR * P),
            "wv": wv.reshape(P, ET * 512),
            "wo": wo.reshape(P, NPAIR * D),
        })

    trace = bool(os.environ.get("BASS_TRACE"))
    res = run_bass_kernel_spmd(nc, in_maps, core_ids=list(range(NCORES)),
                               trace=trace)
    LAST_EXEC_TIME_NS = res.exec_time_ns

    outs = [np.asarray(r["out"], np.float32) for r in res.results]
    return np.stack([outs[2 * b] + outs[2 * b + 1] for b in range(B)])
